# revision 9
# baseline (speedup 1.0000x reference)
"""Chamfer loss kernel for Trainium2 (8 NeuronCores).

Problem: x, y: [4, 3, 8192] f32.  d2[b,n,m] = ||x[b,:,n] - y[b,:,m]||^2.
out = mean_n(min_m d2) + mean_m(min_n d2)  (scalar f32).

Sharding: core c -> batch c//2, point-half c%2.  Each core runs two
symmetric passes (x-side and y-side row-mins over the full opposing set),
so every core's outputs are final mins for a disjoint set of points and
no cross-core reduction is needed.

Device math: one K=6 matmul per (n-tile, m-block) produces
psum[n,m] = y^2[m] - 2*x.y  via weights [-2x0,-2x1,-2x2,1,1,1] and
rhs [y0,y1,y2,y0^2,y1^2,y2^2].  Row-min over m is extracted with fused
tensor_tensor_reduce(min,min) ops; the per-point +x^2[n] and the final
means are O(N) host post-processing.
"""

import sys

if '/opt/trn_rl_repo' not in sys.path:
    sys.path.insert(0, '/opt/trn_rl_repo')

import numpy as np

import concourse.bacc as bacc
import concourse.mybir as mybir
import concourse.tile as tile
from concourse.bass_utils import run_bass_kernel_spmd

import concourse.dve_ops as dve_ops_mod
from concourse.dve_ops import DveOp
from concourse.dve_spec import (Spec, Src0, Src1, C0, minn, lower, AluOp,
                                _has_src1)
from concourse.dve_uop import DveOpSpec

F32 = mybir.dt.float32
BIG = 3.0e38


def _ref_min2(in0, in1, c0, c1, c2):
    b = np.minimum(in0.astype(np.float32), in1.astype(np.float32))
    return b, np.minimum(
        np.asarray(c0, np.float32).reshape(-1, 1) if np.ndim(c0) else np.float32(c0),
        b.reshape(b.shape[0], -1).min(axis=-1, keepdims=True))


def register_min2():
    """Custom DVE op: out = min(in0, in1); accum_out = min(s0, min(out)).

    The standard-ISA TENSOR_TENSOR_REDUCE opcode is not supported by the
    runtime here, but custom-DVE ops ship their own uop table with the NEFF.
    This fused op consumes two 512-wide tiles per instruction (one PSUM, one
    SBUF), which is what keeps the DVE at ~0.5 cycles per reduced column."""
    name = "CHAMFER_MIN2_REDUCE"
    if name in dve_ops_mod._SUB_OPCODE_FOR_NAME:
        return next(op for op in dve_ops_mod.OPS if op.name == name)
    spec = Spec(body=minn(Src0, Src1), accum=AluOp.MIN, accum_init=C0,
                reference=_ref_min2)
    row = dve_ops_mod._CUSTOM_DVE_ROW_BASE + len(dve_ops_mod.OPS)
    dve_ops_mod._SUB_OPCODE_FOR_NAME[name] = row
    shas = {}
    for ver in ("v3", "v4"):
        uops = lower(spec, ver=ver)
        shas[ver] = DveOpSpec(name=name, opcode=row, uops=uops,
                              rd1_en=_has_src1(spec)).sha(ver)
    op = DveOp(name, spec, subdim=False, uops_sha=shas)
    dve_ops_mod.OPS.append(op)
    dve_ops_mod.CUSTOM_DVE_SPECS[name] = spec
    return op


MIN2 = register_min2()

B = 4
C = 3
NPTS = 8192   # points per cloud
NSHARD = NPTS // 2  # points handled per core per side
N_CORES = 8


def _emit_pass(nc, tc, pools, shard_dram, full_dram, out_dram, tag):
    """Emit one pass: for each point p in shard (as weights), compute
    min over the full opposing cloud of (full^2 - 2*shard.full),
    writing [128, NT] mins (partition = point % 128, col = point // 128)."""
    NT = NSHARD // 128       # n-tiles (weight tiles)
    MB = NPTS // 512         # 512-wide m-blocks

    const_pool = pools["const"]
    psum_pool = pools["psum"]
    copy_pool = pools["copy"]
    scratch_pool = pools["scratch"]
    accum_pool = pools["accum"]

    # Weights W: [6, NSHARD] = rows [-2*s0, -2*s1, -2*s2, 1, 1, 1]
    # (compute ops must start at partition 0: memset whole tile to 1.0
    #  first, then overwrite/scale rows 0-2)
    W = const_pool.tile([6, NSHARD], F32, tag=f"W_{tag}")
    nc.gpsimd.memset(W[:], 1.0)
    nc.sync.dma_start(W[0:3, :], shard_dram[:])
    nc.vector.tensor_scalar_mul(W[0:3, :], W[0:3, :], -2.0)

    # Rhs R: [6, NPTS] = rows [f0, f1, f2, f0^2, f1^2, f2^2]
    # squares computed in a partition-0 tile, DMA'd into rows 3-5
    R = const_pool.tile([6, NPTS], F32, tag=f"R_{tag}")
    SQ = const_pool.tile([3, NPTS], F32, tag=f"SQ_{tag}")
    nc.sync.dma_start(R[0:3, :], full_dram[:])
    nc.scalar.activation(SQ[:], R[0:3, :], mybir.ActivationFunctionType.Square)
    nc.sync.dma_start(R[3:6, :], SQ[:])

    minbuf = const_pool.tile([128, NT], F32, tag=f"minbuf_{tag}")

    for t in range(NT):
        wslice = W[:, t * 128:(t + 1) * 128]
        accum = accum_pool.tile([128, MB // 2], F32, tag="acc")
        for i in range(MB // 2):
            pa = psum_pool.tile([128, 512], F32, tag="ps")
            nc.tensor.matmul(pa[:], wslice,
                             R[:, (2 * i) * 512:(2 * i + 1) * 512],
                             start=True, stop=True)
            pb = psum_pool.tile([128, 512], F32, tag="ps")
            nc.tensor.matmul(pb[:], wslice,
                             R[:, (2 * i + 1) * 512:(2 * i + 2) * 512],
                             start=True, stop=True)
            cp = copy_pool.tile([128, 512], F32, tag="cp")
            nc.scalar.copy(cp[:], pb[:])
            scr = scratch_pool.tile([128, 512], F32, tag="scr")
            nc.vector._custom_dve(MIN2, out=scr[:], in0=pa[:], in1=cp[:],
                                  s0=BIG, accum_out=accum[:, i:i + 1])
        nc.vector.tensor_reduce(minbuf[:, t:t + 1], accum[:],
                                axis=mybir.AxisListType.X,
                                op=mybir.AluOpType.min)

    nc.sync.dma_start(out_dram[:], minbuf[:])


def build_program():
    from contextlib import ExitStack
    nc = bacc.Bacc("TRN2", target_bir_lowering=False, debug=False)
    NT = NSHARD // 128

    xs = nc.dram_tensor("xs", [C, NSHARD], F32, kind="ExternalInput")
    yf = nc.dram_tensor("yf", [C, NPTS], F32, kind="ExternalInput")
    ys = nc.dram_tensor("ys", [C, NSHARD], F32, kind="ExternalInput")
    xf = nc.dram_tensor("xf", [C, NPTS], F32, kind="ExternalInput")
    minx = nc.dram_tensor("minx", [128, NT], F32, kind="ExternalOutput")
    miny = nc.dram_tensor("miny", [128, NT], F32, kind="ExternalOutput")

    with tile.TileContext(nc) as tc:
        with ExitStack() as ctx:
            pools = {
                "const": ctx.enter_context(tc.tile_pool(name="const", bufs=1)),
                "psum": ctx.enter_context(
                    tc.tile_pool(name="psum", bufs=8, space="PSUM")),
                "copy": ctx.enter_context(tc.tile_pool(name="copy", bufs=4)),
                "scratch": ctx.enter_context(tc.tile_pool(name="scr", bufs=2)),
                "accum": ctx.enter_context(tc.tile_pool(name="acc", bufs=2)),
            }
            _emit_pass(nc, tc, pools, xs, yf, minx, "a")
            _emit_pass(nc, tc, pools, ys, xf, miny, "b")
    nc.compile()
    return nc


_cached_nc = None


def _get_nc():
    global _cached_nc
    if _cached_nc is None:
        _cached_nc = build_program()
    return _cached_nc


def run_sharded(x, y, trace=False, **kw):
    """Returns (scalar_out, BassKernelResults)."""
    x = np.ascontiguousarray(x, dtype=np.float32)
    y = np.ascontiguousarray(y, dtype=np.float32)
    nc = _get_nc()
    in_maps = []
    for c in range(N_CORES):
        b, h = c // 2, c % 2
        sl = slice(h * NSHARD, (h + 1) * NSHARD)
        in_maps.append({
            "xs": np.ascontiguousarray(x[b, :, sl]),
            "yf": np.ascontiguousarray(y[b]),
            "ys": np.ascontiguousarray(y[b, :, sl]),
            "xf": np.ascontiguousarray(x[b]),
        })
    res = run_bass_kernel_spmd(nc, in_maps, core_ids=list(range(N_CORES)),
                               trace=trace, **kw)

    # Host epilogue: add ||p||^2 for each sharded point, then mean.
    x2 = np.sum(x.astype(np.float64) ** 2, axis=1)  # [B, NPTS]
    y2 = np.sum(y.astype(np.float64) ** 2, axis=1)  # [B, NPTS]
    sx = 0.0
    sy = 0.0
    for c in range(N_CORES):
        b, h = c // 2, c % 2
        sl = slice(h * NSHARD, (h + 1) * NSHARD)
        vx = res.results[c]["minx"].T.reshape(-1).astype(np.float64)
        vy = res.results[c]["miny"].T.reshape(-1).astype(np.float64)
        sx += np.sum(vx + x2[b, sl])
        sy += np.sum(vy + y2[b, sl])
    out = np.float32(sx / (B * NPTS) + sy / (B * NPTS))
    return out, res


def kernel(x, y):
    out, _ = run_sharded(x, y, trace=False)
    return out


# revision 10
# speedup vs baseline: 3.8990x; 3.8990x over previous
"""Chamfer loss kernel for Trainium2 (8 NeuronCores).

Problem: x, y: [4, 3, 8192] f32.  d2[b,n,m] = ||x[b,:,n] - y[b,:,m]||^2.
out = mean_n(min_m d2) + mean_m(min_n d2)  (scalar f32).

Sharding: core c -> batch c//2, point-half c%2.  Each core runs two
symmetric passes (x-side and y-side row-mins over the full opposing
cloud), so every core's outputs are final mins for a disjoint set of
points and no cross-core reduction is needed.

Device math: one K=15 bf16 matmul per (n-tile, m-block) produces
psum[n,m] = y^2[m] - 2*x.y  (to ~2^-18 relative) via hi/lo split rows:

  k 0..2:   W=-2*xh_d  R=yh_d        k 9..11:  W=1  R=hi(y_d^2)
  k 3..5:   W=-2*xl_d  R=yh_d        k 12..14: W=1  R=lo(y_d^2)
  k 6..8:   W=-2*xh_d  R=yl_d

bf16 products are exact in f32 PSUM; only the xl*yl term (~2^-18) is
dropped.  fp32 matmuls would be ~5x slower on the PE (hi/lo double
pass at half stream rate).

Row-min over m is extracted with a custom fused DVE op
(min(in0,in1) + min-accumulate) that consumes one PSUM tile and one
ScalarE-copied SBUF tile per instruction.  The per-point +x^2[n] and
final means are O(N) host post-processing, as is building the split
rows (host numpy, O(N)).
"""

import sys

if '/opt/trn_rl_repo' not in sys.path:
    sys.path.insert(0, '/opt/trn_rl_repo')

import ml_dtypes
import numpy as np

import concourse.bacc as bacc
import concourse.mybir as mybir
import concourse.tile as tile
from concourse.bass_utils import run_bass_kernel_spmd

import concourse.dve_ops as dve_ops_mod
from concourse.dve_ops import DveOp
from concourse.dve_spec import (Spec, Src0, Src1, C0, minn, lower, AluOp,
                                _has_src1)
from concourse.dve_uop import DveOpSpec

F32 = mybir.dt.float32
BF16 = mybir.dt.bfloat16
NPBF16 = ml_dtypes.bfloat16
BIG = 3.0e38

B = 4
C = 3
K = 15        # split-K augmented contraction dim
NPTS = 8192   # points per cloud
NSHARD = NPTS // 2  # points handled per core per side
N_CORES = 8


def _ref_min2(in0, in1, c0, c1, c2):
    b = np.minimum(in0.astype(np.float32), in1.astype(np.float32))
    return b, np.minimum(
        np.asarray(c0, np.float32).reshape(-1, 1) if np.ndim(c0) else np.float32(c0),
        b.reshape(b.shape[0], -1).min(axis=-1, keepdims=True))


def register_min2():
    """Custom DVE op: out = min(in0, in1); accum_out = min(s0, min(out)).

    The standard-ISA TENSOR_TENSOR_REDUCE opcode is not supported by the
    runtime here, but custom-DVE ops ship their own uop table with the NEFF.
    This fused op consumes two 512-wide tiles per instruction (one PSUM, one
    SBUF), which is what keeps the DVE at ~0.75 cycles per reduced column."""
    name = "CHAMFER_MIN2_REDUCE"
    if name in dve_ops_mod._SUB_OPCODE_FOR_NAME:
        return next(op for op in dve_ops_mod.OPS if op.name == name)
    spec = Spec(body=minn(Src0, Src1), accum=AluOp.MIN, accum_init=C0,
                reference=_ref_min2)
    row = dve_ops_mod._CUSTOM_DVE_ROW_BASE + len(dve_ops_mod.OPS)
    dve_ops_mod._SUB_OPCODE_FOR_NAME[name] = row
    shas = {}
    for ver in ("v3", "v4"):
        uops = lower(spec, ver=ver)
        shas[ver] = DveOpSpec(name=name, opcode=row, uops=uops,
                              rd1_en=_has_src1(spec)).sha(ver)
    op = DveOp(name, spec, subdim=False, uops_sha=shas)
    dve_ops_mod.OPS.append(op)
    dve_ops_mod.CUSTOM_DVE_SPECS[name] = spec
    return op


MIN2 = register_min2()


def _emit_pass(nc, tc, pools, w_dram, r_dram, out_dram, tag):
    """One pass: W [K, NSHARD] bf16 weight rows, R [K, NPTS] bf16 rhs rows,
    out [128, NT] f32 row-mins (partition = point % 128, col = point//128)."""
    NT = NSHARD // 128       # weight tiles
    MB = NPTS // 512         # 512-wide m-blocks

    const_pool = pools["const"]
    psum_pool = pools["psum"]
    copy_pool = pools["copy"]
    scratch_pool = pools["scratch"]
    accum_pool = pools["accum"]

    W = const_pool.tile([K, NSHARD], BF16, tag=f"W_{tag}")
    nc.sync.dma_start(W[:], w_dram[:])
    R = const_pool.tile([K, NPTS], BF16, tag=f"R_{tag}")
    nc.sync.dma_start(R[:], r_dram[:])

    minbuf = const_pool.tile([128, NT], F32, tag=f"minbuf_{tag}")

    for t in range(NT):
        wslice = W[:, t * 128:(t + 1) * 128]
        accum = accum_pool.tile([128, MB // 2], F32, tag="acc")
        for i in range(MB // 2):
            pa = psum_pool.tile([128, 512], F32, tag="ps")
            nc.tensor.matmul(pa[:], wslice,
                             R[:, (2 * i) * 512:(2 * i + 1) * 512],
                             start=True, stop=True)
            pb = psum_pool.tile([128, 512], F32, tag="ps")
            nc.tensor.matmul(pb[:], wslice,
                             R[:, (2 * i + 1) * 512:(2 * i + 2) * 512],
                             start=True, stop=True)
            cp = copy_pool.tile([128, 512], F32, tag="cp")
            nc.scalar.copy(cp[:], pb[:])
            scr = scratch_pool.tile([128, 512], F32, tag="scr")
            nc.vector._custom_dve(MIN2, out=scr[:], in0=pa[:], in1=cp[:],
                                  s0=BIG, accum_out=accum[:, i:i + 1])
        nc.vector.tensor_reduce(minbuf[:, t:t + 1], accum[:],
                                axis=mybir.AxisListType.X,
                                op=mybir.AluOpType.min)

    nc.sync.dma_start(out_dram[:], minbuf[:])


def build_program():
    from contextlib import ExitStack
    nc = bacc.Bacc("TRN2", target_bir_lowering=False, debug=False)
    NT = NSHARD // 128

    wa = nc.dram_tensor("wa", [K, NSHARD], BF16, kind="ExternalInput")
    ra = nc.dram_tensor("ra", [K, NPTS], BF16, kind="ExternalInput")
    wb = nc.dram_tensor("wb", [K, NSHARD], BF16, kind="ExternalInput")
    rb = nc.dram_tensor("rb", [K, NPTS], BF16, kind="ExternalInput")
    minx = nc.dram_tensor("minx", [128, NT], F32, kind="ExternalOutput")
    miny = nc.dram_tensor("miny", [128, NT], F32, kind="ExternalOutput")

    with tile.TileContext(nc) as tc:
        with ExitStack() as ctx:
            pools = {
                "const": ctx.enter_context(tc.tile_pool(name="const", bufs=1)),
                "psum": ctx.enter_context(
                    tc.tile_pool(name="psum", bufs=8, space="PSUM")),
                "copy": ctx.enter_context(tc.tile_pool(name="copy", bufs=4)),
                "scratch": ctx.enter_context(tc.tile_pool(name="scr", bufs=2)),
                "accum": ctx.enter_context(tc.tile_pool(name="acc", bufs=2)),
            }
            _emit_pass(nc, tc, pools, wa, ra, minx, "a")
            _emit_pass(nc, tc, pools, wb, rb, miny, "b")
    nc.compile()
    return nc


_cached_nc = None


def _get_nc():
    global _cached_nc
    if _cached_nc is None:
        _cached_nc = build_program()
    return _cached_nc


def _split_w(shard):
    """shard: [3, n] f32 -> [K, n] bf16 weight rows."""
    n = shard.shape[1]
    xh = shard.astype(NPBF16)
    xl = (shard - xh.astype(np.float32)).astype(NPBF16)
    w = np.empty((K, n), NPBF16)
    w[0:3] = (-2.0 * xh.astype(np.float32)).astype(NPBF16)   # exact scale
    w[3:6] = (-2.0 * xl.astype(np.float32)).astype(NPBF16)
    w[6:9] = w[0:3]
    w[9:15] = NPBF16(1.0)
    return w


def _split_r(full):
    """full: [3, m] f32 -> [K, m] bf16 rhs rows."""
    m = full.shape[1]
    yh = full.astype(NPBF16)
    yl = (full - yh.astype(np.float32)).astype(NPBF16)
    sq = (full.astype(np.float32) ** 2)
    sqh = sq.astype(NPBF16)
    sql = (sq - sqh.astype(np.float32)).astype(NPBF16)
    r = np.empty((K, m), NPBF16)
    r[0:3] = yh
    r[3:6] = yh
    r[6:9] = yl
    r[9:12] = sqh
    r[12:15] = sql
    return r


def run_sharded(x, y, trace=False, **kw):
    """Returns (scalar_out, BassKernelResults)."""
    x = np.ascontiguousarray(x, dtype=np.float32)
    y = np.ascontiguousarray(y, dtype=np.float32)
    nc = _get_nc()
    in_maps = []
    for c in range(N_CORES):
        b, h = c // 2, c % 2
        sl = slice(h * NSHARD, (h + 1) * NSHARD)
        in_maps.append({
            "wa": _split_w(x[b, :, sl]),
            "ra": _split_r(y[b]),
            "wb": _split_w(y[b, :, sl]),
            "rb": _split_r(x[b]),
        })
    res = run_bass_kernel_spmd(nc, in_maps, core_ids=list(range(N_CORES)),
                               trace=trace, **kw)

    # Host epilogue: add ||p||^2 for each sharded point, then mean.
    x2 = np.sum(x.astype(np.float64) ** 2, axis=1)  # [B, NPTS]
    y2 = np.sum(y.astype(np.float64) ** 2, axis=1)  # [B, NPTS]
    sx = 0.0
    sy = 0.0
    for c in range(N_CORES):
        b, h = c // 2, c % 2
        sl = slice(h * NSHARD, (h + 1) * NSHARD)
        vx = res.results[c]["minx"].T.reshape(-1).astype(np.float64)
        vy = res.results[c]["miny"].T.reshape(-1).astype(np.float64)
        sx += np.sum(vx + x2[b, sl])
        sy += np.sum(vy + y2[b, sl])
    out = np.float32(sx / (B * NPTS) + sy / (B * NPTS))
    return out, res


def kernel(x, y):
    out, _ = run_sharded(x, y, trace=False)
    return out
